# revision 35
# baseline (speedup 1.0000x reference)
"""Multi-head attention (q/k/v projections + softmax attention + out-projection)
on 8 Trainium2 NeuronCores.

Sharding: 16 (batch, head) units over 8 cores -> core c handles batch n = c//4
and head pair hp = c%4 (columns 128*hp : 128*hp+128 of the projections).
Per-core partial outputs (each pair's contribution to mix @ Wo) are summed on
host per batch, + bo.

Device kernel (per core), v6:
  - Host pre-transposes q[n],k[n],v[n] -> xT [512, 4096] and converts x and
    Wq/Wk/Wv to bf16 (halves input DMA; bf16 rhs streams 1 cy/row on the PE
    at any N, fixing the fp32r N=128 4x penalty on the V-projection).
  - Projections are interleaved into the first lq-window's attention blocks,
    so ScalarE exp starts a few us in instead of idling through a serial
    projection phase.
  - Attention runs on 512-wide lq windows (8 of them). Per window the two
    PV accumulators are [128,512] = 1 PSUM bank each, which frees budget
    for THREE [128, 2(kv),512] S^T tiles: the S->exp->free round-trip is
    no longer the pipeline pacer (with 2 buffers it was).
  - S^T in fp32r (K=64); exp on ScalarE over [128, 2x512] tiles (one per
    kv-chunk-pair and head); PV in bf16 accumulates mixT[c,lq] + sum(exp)
    via a ones-column in VP (transpose-free). PV runs one pair behind exp
    (software pipelining) so it never blocks the S/exp chain.
  - est tiles for (h1, odd kv-pair) in lq windows 1-7 (~22% of all tiles)
    are computed on the DVE instead via a Schraudolph fast-exp
    (bitcast_bf16(int16(x*c1+c2)), one tensor_scalar op) - whole-tile
    offload spreads the approximation across kv so softmax dilutes it;
    this keeps ScalarE below the PE's pace.
  - softmax normalization: 1/sum(exp) from the PSUM row on DVE, GPSIMD
    partition-broadcast, one DVE multiply folds normalization into the
    PSUM->SBUF move of mix.
  - Out-projection is deferred to a tail phase: mix rows are h0|h1 c-dims,
    so one K=128 matmul against Wo sums both heads; output DMA'd as bf16
    (host sums partials in f32).
"""

import numpy as np
import ml_dtypes

import concourse.bacc as bacc
import concourse.mybir as mybir
import concourse.tile as tile
from concourse import bass_utils

P = 128
L = 4096
D = 512
F32 = mybir.dt.float32
F32R = mybir.dt.float32r
BF16 = mybir.dt.bfloat16
I16 = mybir.dt.int16
AF = mybir.ActivationFunctionType
ALU = mybir.AluOpType

_NC = None
V_OFFL_LQC = 1   # DVE fast-exp for h1/odd pairs in lq windows >= this


def build():
    nc = bacc.Bacc("TRN2", target_bir_lowering=False, debug=False)

    xqt = nc.dram_tensor("xqt", (D, L), BF16, kind="ExternalInput").ap()
    xkt = nc.dram_tensor("xkt", (D, L), BF16, kind="ExternalInput").ap()
    xvt = nc.dram_tensor("xvt", (D, L), BF16, kind="ExternalInput").ap()
    wq = nc.dram_tensor("wq", (D, P), BF16, kind="ExternalInput").ap()
    wk = nc.dram_tensor("wk", (D, P), BF16, kind="ExternalInput").ap()
    wv = nc.dram_tensor("wv", (D, P), BF16, kind="ExternalInput").ap()
    wo = nc.dram_tensor("wo", (P, D), F32R, kind="ExternalInput").ap()
    bqs = nc.dram_tensor("bqs", (P, 1), F32, kind="ExternalInput").ap()
    bkc = nc.dram_tensor("bkc", (P, 1), F32, kind="ExternalInput").ap()
    bvr = nc.dram_tensor("bvr", (1, P), BF16, kind="ExternalInput").ap()
    out = nc.dram_tensor("out", (L, D), BF16, kind="ExternalOutput").ap()

    with tile.TileContext(nc) as tc:
        with tc.tile_pool(name="const", bufs=1) as const, \
             tc.tile_pool(name="persist", bufs=1) as persist:
            xqv = xqt.rearrange("(o p) l -> p o l", p=P)
            xkv = xkt.rearrange("(o p) l -> p o l", p=P)
            xvv = xvt.rearrange("(o p) l -> p o l", p=P)

            wk_sb = const.tile([P, 4, P], BF16, tag="wk")
            nc.sync.dma_start(wk_sb, wk.rearrange("(o p) m -> p o m", p=P))
            wq_sb = const.tile([P, 4, P], BF16, tag="wq")
            nc.sync.dma_start(wq_sb, wq.rearrange("(o p) m -> p o m", p=P))
            bq_sb = const.tile([P, 1], F32, tag="bq")
            nc.sync.dma_start(bq_sb, bqs)
            bk_sb = const.tile([P, 1], F32, tag="bk")
            nc.sync.dma_start(bk_sb, bkc)
            wv_sb = const.tile([P, 4, P], BF16, tag="wv")
            nc.sync.dma_start(wv_sb, wv.rearrange("(o p) m -> p o m", p=P))
            bvr_sb = const.tile([1, P], BF16, tag="bvr")
            nc.sync.dma_start(bvr_sb, bvr)
            onesr = const.tile([1, P], BF16, tag="onesr")
            nc.scalar.activation(onesr, bvr_sb, AF.Identity,
                                 bias=1.0, scale=0.0)

            qpt_t = [persist.tile([P, 512], F32R, tag=f"qpt{c}",
                                  name=f"qpt{c}") for c in range(8)]
            kpt_t = [persist.tile([P, 512], F32R, tag=f"kpt{c}",
                                  name=f"kpt{c}") for c in range(8)]
            vp_t = [persist.tile([P, 4, 130], BF16, tag=f"vp{c}",
                                 name=f"vp{c}") for c in range(8)]
            mix_t = [persist.tile([P, 512], F32R, tag=f"mix{l}",
                                  name=f"mix{l}") for l in range(8)]
            ones_in = bq_sb[:, :, None].to_broadcast((P, 4, 1))
            for c in range(8):
                nc.vector.tensor_scalar(vp_t[c][:, :, 64:65], ones_in,
                                        0.0, 1.0, ALU.mult, ALU.add)
                nc.vector.tensor_scalar(vp_t[c][:, :, 129:130], ones_in,
                                        0.0, 1.0, ALU.mult, ALU.add)

            with tc.tile_pool(name="xs", bufs=2) as xs, \
                 tc.tile_pool(name="psp", bufs=1, space="PSUM") as psp, \
                 tc.tile_pool(name="esp", bufs=6) as esp, \
                 tc.tile_pool(name="bcp", bufs=1) as bcp, \
                 tc.tile_pool(name="obp", bufs=2) as obp:
                stc = [0]  # st tag rotation counter (3 tags x 2 banks)

                def st_tile(shape, name):
                    t = psp.tile(shape, F32, tag=f"st{stc[0] % 3}", name=name)
                    stc[0] += 1
                    return t

                def proj_k(ch):
                    sl = slice(ch * 512, (ch + 1) * 512)
                    xtk = xs.tile([P, 4, 512], BF16, tag="xtk")
                    nc.sync.dma_start(xtk, xkv[:, :, sl])
                    kps = st_tile([P, 512], "kps")
                    for dk in range(4):
                        nc.tensor.matmul(kps, lhsT=wk_sb[:, dk, :],
                                         rhs=xtk[:, dk, :],
                                         start=(dk == 0), stop=(dk == 3))
                    nc.vector.tensor_scalar(kpt_t[ch][:], kps, 1.0, bk_sb,
                                            ALU.mult, ALU.add)

                def proj_v(ch):
                    sl = slice(ch * 512, (ch + 1) * 512)
                    xtv = xs.tile([P, 4, 512], BF16, tag="xtv")
                    nc.gpsimd.dma_start(xtv, xvv[:, :, sl])
                    for js in range(4):
                        psv = st_tile([P, P], "psv")
                        for dk in range(4):
                            nc.tensor.matmul(psv,
                                             lhsT=xtv[:, dk, js * P:(js + 1) * P],
                                             rhs=wv_sb[:, dk, :],
                                             start=(dk == 0), stop=False)
                        nc.tensor.matmul(psv, lhsT=onesr, rhs=bvr_sb,
                                         start=False, stop=True)
                        nc.vector.tensor_copy(vp_t[ch][:, js, 0:64],
                                              psv[:, 0:64])
                        nc.vector.tensor_copy(vp_t[ch][:, js, 65:129],
                                              psv[:, 64:128])

                def proj_q(ch):
                    sl = slice(ch * 512, (ch + 1) * 512)
                    xtq = xs.tile([P, 4, 512], BF16, tag="xtq")
                    nc.gpsimd.dma_start(xtq, xqv[:, :, sl])
                    qps = st_tile([P, 512], "qps")
                    for dk in range(4):
                        nc.tensor.matmul(qps, lhsT=wq_sb[:, dk, :],
                                         rhs=xtq[:, dk, :],
                                         start=(dk == 0), stop=(dk == 3))
                    nc.vector.tensor_scalar(qpt_t[ch][:], qps, 0.125, bq_sb,
                                            ALU.mult, ALU.add)

                # prologue: K/Q only, so the first S tile isn't queued
                # behind V-projection matmuls waiting on their DMA
                proj_k(0)
                proj_q(0)
                proj_k(1)
                wo_sb = const.tile([P, D], F32R, tag="wo")
                nc.sync.dma_start(wo_sb, wo)

                # Schraudolph fast-exp in bf16: exp(x) ~=
                # bitcast_bf16(int16(x * 2^7/ln2 + (127*2^7 - 7.42)))
                SC1, SC2 = 184.6650292, 16249.0

                def emit_pv(pair, ests, pv_ps):
                    for h in range(2):
                        for r in range(2):
                            j = 2 * pair + r
                            nc.tensor.matmul(
                                pv_ps[h][0:65, :],
                                lhsT=vp_t[j // 4][:, j % 4,
                                                  h * 65:(h + 1) * 65],
                                rhs=ests[h][:, r, :],
                                start=(j == 0), stop=(j == 31))

                for lqc in range(8):
                    pv_ps = [psp.tile([P, 512], F32, tag=f"pv{h}",
                                      name=f"pv{h}") for h in range(2)]
                    prev = None
                    for b in range(8):
                        for pair in range(2 * b, 2 * b + 2):
                            ests = []
                            for h in range(2):
                                hb = h * 64
                                st = st_tile([P, 2, 512], "st")
                                for r in range(2):
                                    j = 2 * pair + r
                                    nc.tensor.matmul(
                                        st[:, r, :],
                                        lhsT=kpt_t[j // 4][hb:hb + 64,
                                                           (j % 4) * P:(j % 4 + 1) * P],
                                        rhs=qpt_t[lqc][hb:hb + 64, :],
                                        start=True, stop=True)
                                est = esp.tile([P, 2, 512], BF16,
                                               tag=f"est{h}")
                                if (lqc >= V_OFFL_LQC and h == 1
                                        and pair % 2 == 1):
                                    nc.vector.tensor_scalar(
                                        est[:, :, :].bitcast(I16),
                                        st, SC1, SC2, ALU.mult, ALU.add)
                                else:
                                    nc.scalar.activation(est, st, AF.Exp)
                                ests.append(est)
                            if prev is not None:
                                emit_pv(prev[0], prev[1], pv_ps)
                            prev = (pair, ests)
                            # interleaved projections, half-chunk granular:
                            # v(c) at pair 2c-1 (v0 at 0), k(c) at 2(c-2)
                            if lqc == 0 and pair < 14:
                                if pair == 0:
                                    proj_v(0)
                                elif pair % 2 == 1:
                                    proj_v((pair + 1) // 2)
                                else:
                                    proj_k(pair // 2 + 1)
                            elif lqc == 0 and pair == 14:
                                proj_q(1)
                            elif lqc == 0 and pair == 15:
                                proj_q(2)
                        if lqc in (1, 2) and b in (0, 4):
                            qch = 2 * lqc + 1 + (b // 4)
                            if 2 < qch < 8:
                                proj_q(qch)
                        if lqc == 3 and b == 0:
                            proj_q(7)
                    emit_pv(prev[0], prev[1], pv_ps)
                    # free pv banks fast: raw copy to SBUF; normalization
                    # (recip/broadcast/mult into mix) happens lazily and
                    # overlaps the next window's attention.
                    for h in range(2):
                        raw = bcp.tile([P, 512], F32, tag=f"raw{h}", bufs=2)
                        nc.vector.tensor_copy(raw[0:65, :], pv_ps[h][0:65, :])
                        rr = bcp.tile([1, 512], F32, tag=f"rr{h}", bufs=2)
                        nc.vector.reciprocal(rr, raw[64:65, :])
                        bc = bcp.tile([P, 512], F32, tag=f"bc{h}", bufs=2)
                        nc.gpsimd.partition_broadcast(bc, rr)
                        nc.vector.tensor_tensor(
                            mix_t[lqc][h * 64:(h + 1) * 64, :],
                            raw[0:64, :], bc[0:64, :], ALU.mult)

                # tail: out-projection. mix rows 0:64 = h0 c-dims,
                # 64:128 = h1 c-dims, so a single K=128 matmul against
                # wo_sb sums both heads' contributions.
                tags = ["st0", "st1", "st2", "pv0", "pv1"]
                for lqc in range(8):
                    for s in range(4):
                        i = lqc * 4 + s
                        ops = psp.tile([P, D], F32, tag=tags[i % 5],
                                       name="ops")
                        nc.tensor.matmul(
                            ops, lhsT=mix_t[lqc][:, s * P:(s + 1) * P],
                            rhs=wo_sb, start=True, stop=True)
                        ob = obp.tile([P, D], BF16, tag=f"ob{i % 3}")
                        if i % 2 == 0:
                            nc.scalar.copy(ob, ops)
                        else:
                            nc.vector.tensor_copy(ob, ops)
                        nc.sync.dma_start(
                            out[lqc * 512 + s * P:lqc * 512 + (s + 1) * P, :],
                            ob)

    nc.compile()
    return nc


def get_nc():
    global _NC
    if _NC is None:
        _NC = build()
    return _NC


def make_in_maps(q, k, v, Wq, bq, Wk, bk, Wv, bv, Wo, bo):
    bf = ml_dtypes.bfloat16
    q = np.asarray(q, np.float32)
    k = np.asarray(k, np.float32)
    v = np.asarray(v, np.float32)
    Wq = np.asarray(Wq, np.float32)
    Wk = np.asarray(Wk, np.float32)
    Wv = np.asarray(Wv, np.float32)
    Wo = np.asarray(Wo, np.float32)
    bq = np.asarray(bq, np.float32)
    bk = np.asarray(bk, np.float32)
    bv = np.asarray(bv, np.float32)
    xts = {}
    for n in range(2):
        xts[n] = (np.ascontiguousarray(q[n].T).astype(bf),
                  np.ascontiguousarray(k[n].T).astype(bf),
                  np.ascontiguousarray(v[n].T).astype(bf))
    in_maps = []
    for c in range(8):
        n, hp = c // 4, c % 4
        sl = slice(P * hp, P * (hp + 1))
        xq, xk, xv = xts[n]
        in_maps.append({
            "xqt": xq, "xkt": xk, "xvt": xv,
            "wq": np.ascontiguousarray(Wq[:, sl]).astype(bf),
            "wk": np.ascontiguousarray(Wk[:, sl]).astype(bf),
            "wv": np.ascontiguousarray(Wv[:, sl]).astype(bf),
            "wo": np.ascontiguousarray(Wo[sl, :]),
            "bqs": (bq[sl] * 0.125).reshape(P, 1).astype(np.float32),
            "bkc": bk[sl].reshape(P, 1).astype(np.float32),
            "bvr": bv[sl].reshape(1, P).astype(bf),
        })
    return in_maps


def assemble(results, bo):
    bo = np.asarray(bo, np.float32)
    out = np.zeros((2, L, D), np.float32)
    for c in range(8):
        out[c // 4] += np.asarray(results[c]["out"], dtype=np.float32)
    out += bo[None, None, :]
    return out


def kernel(q, k, v, Wq, bq, Wk, bk, Wv, bv, Wo, bo):
    nc = get_nc()
    in_maps = make_in_maps(q, k, v, Wq, bq, Wk, bk, Wv, bv, Wo, bo)
    res = bass_utils.run_bass_kernel_spmd(nc, in_maps, core_ids=list(range(8)))
    return assemble(res.results, bo)


if __name__ == "__main__":
    build()
    print("build ok")


# revision 36
# speedup vs baseline: 1.0061x; 1.0061x over previous
"""Multi-head attention (q/k/v projections + softmax attention + out-projection)
on 8 Trainium2 NeuronCores.

Sharding: 16 (batch, head) units over 8 cores -> core c handles batch n = c//4
and head pair hp = c%4 (columns 128*hp : 128*hp+128 of the projections).
Per-core partial outputs (each pair's contribution to mix @ Wo) are summed on
host per batch, + bo.

Device kernel (per core), v6:
  - Host pre-transposes q[n],k[n],v[n] -> xT [512, 4096] and converts x and
    Wq/Wk/Wv to bf16 (halves input DMA; bf16 rhs streams 1 cy/row on the PE
    at any N, fixing the fp32r N=128 4x penalty on the V-projection).
  - Projections are interleaved into the first lq-window's attention blocks,
    so ScalarE exp starts a few us in instead of idling through a serial
    projection phase.
  - Attention runs on 512-wide lq windows (8 of them). Per window the two
    PV accumulators are [128,512] = 1 PSUM bank each, which frees budget
    for THREE [128, 2(kv),512] S^T tiles: the S->exp->free round-trip is
    no longer the pipeline pacer (with 2 buffers it was).
  - S^T in fp32r (K=64); exp on ScalarE over [128, 2x512] tiles (one per
    kv-chunk-pair and head); PV in bf16 accumulates mixT[c,lq] + sum(exp)
    via a ones-column in VP (transpose-free). PV runs one pair behind exp
    (software pipelining) so it never blocks the S/exp chain.
  - est tiles for (h1, odd kv-pair) in lq windows 1-7 (~22% of all tiles)
    are computed on the DVE instead via a Schraudolph fast-exp
    (bitcast_bf16(int16(x*c1+c2)), one tensor_scalar op) - whole-tile
    offload spreads the approximation across kv so softmax dilutes it;
    this keeps ScalarE below the PE's pace.
  - softmax normalization: 1/sum(exp) from the PSUM row on DVE, GPSIMD
    partition-broadcast, one DVE multiply folds normalization into the
    PSUM->SBUF move of mix.
  - Out-projection is deferred to a tail phase: mix rows are h0|h1 c-dims,
    so one K=128 matmul against Wo sums both heads; output DMA'd as bf16
    (host sums partials in f32).
"""

import numpy as np
import ml_dtypes

import concourse.bacc as bacc
import concourse.mybir as mybir
import concourse.tile as tile
from concourse import bass_utils

P = 128
L = 4096
D = 512
F32 = mybir.dt.float32
F32R = mybir.dt.float32r
BF16 = mybir.dt.bfloat16
I16 = mybir.dt.int16
AF = mybir.ActivationFunctionType
ALU = mybir.AluOpType

_NC = None
V_OFFL_LQC = 1   # DVE fast-exp for h1/odd pairs in lq windows >= this


def build():
    nc = bacc.Bacc("TRN2", target_bir_lowering=False, debug=False)

    xqt = nc.dram_tensor("xqt", (D, L), BF16, kind="ExternalInput").ap()
    xkt = nc.dram_tensor("xkt", (D, L), BF16, kind="ExternalInput").ap()
    xvt = nc.dram_tensor("xvt", (D, L), BF16, kind="ExternalInput").ap()
    wq = nc.dram_tensor("wq", (D, P), BF16, kind="ExternalInput").ap()
    wk = nc.dram_tensor("wk", (D, P), BF16, kind="ExternalInput").ap()
    wv = nc.dram_tensor("wv", (D, P), BF16, kind="ExternalInput").ap()
    wo = nc.dram_tensor("wo", (P, D), F32R, kind="ExternalInput").ap()
    bqs = nc.dram_tensor("bqs", (P, 1), F32, kind="ExternalInput").ap()
    bkc = nc.dram_tensor("bkc", (P, 1), F32, kind="ExternalInput").ap()
    bvr = nc.dram_tensor("bvr", (1, P), BF16, kind="ExternalInput").ap()
    out = nc.dram_tensor("out", (L, D), BF16, kind="ExternalOutput").ap()

    with tile.TileContext(nc) as tc:
        with tc.tile_pool(name="const", bufs=1) as const, \
             tc.tile_pool(name="persist", bufs=1) as persist:
            xqv = xqt.rearrange("(o p) l -> p o l", p=P)
            xkv = xkt.rearrange("(o p) l -> p o l", p=P)
            xvv = xvt.rearrange("(o p) l -> p o l", p=P)

            wk_sb = const.tile([P, 4, P], BF16, tag="wk")
            nc.sync.dma_start(wk_sb, wk.rearrange("(o p) m -> p o m", p=P))
            wq_sb = const.tile([P, 4, P], BF16, tag="wq")
            nc.sync.dma_start(wq_sb, wq.rearrange("(o p) m -> p o m", p=P))
            bq_sb = const.tile([P, 1], F32, tag="bq")
            nc.sync.dma_start(bq_sb, bqs)
            bk_sb = const.tile([P, 1], F32, tag="bk")
            nc.sync.dma_start(bk_sb, bkc)
            wv_sb = const.tile([P, 4, P], BF16, tag="wv")
            nc.sync.dma_start(wv_sb, wv.rearrange("(o p) m -> p o m", p=P))
            bvr_sb = const.tile([1, P], BF16, tag="bvr")
            nc.sync.dma_start(bvr_sb, bvr)
            onesr = const.tile([1, P], BF16, tag="onesr")
            nc.scalar.activation(onesr, bvr_sb, AF.Identity,
                                 bias=1.0, scale=0.0)

            qpt_t = [persist.tile([P, 512], F32R, tag=f"qpt{c}",
                                  name=f"qpt{c}") for c in range(8)]
            kpt_t = [persist.tile([P, 512], F32R, tag=f"kpt{c}",
                                  name=f"kpt{c}") for c in range(8)]
            vp_t = [persist.tile([P, 4, 130], BF16, tag=f"vp{c}",
                                 name=f"vp{c}") for c in range(8)]
            mix_t = [persist.tile([P, 512], F32R, tag=f"mix{l}",
                                  name=f"mix{l}") for l in range(8)]
            ones_in = bq_sb[:, :, None].to_broadcast((P, 4, 1))
            for c in range(8):
                nc.vector.tensor_scalar(vp_t[c][:, :, 64:65], ones_in,
                                        0.0, 1.0, ALU.mult, ALU.add)
                nc.vector.tensor_scalar(vp_t[c][:, :, 129:130], ones_in,
                                        0.0, 1.0, ALU.mult, ALU.add)

            with tc.tile_pool(name="xs", bufs=2) as xs, \
                 tc.tile_pool(name="psp", bufs=1, space="PSUM") as psp, \
                 tc.tile_pool(name="esp", bufs=6) as esp, \
                 tc.tile_pool(name="bcp", bufs=1) as bcp, \
                 tc.tile_pool(name="obp", bufs=2) as obp:
                stc = [0]  # st tag rotation counter (3 tags x 2 banks)

                def st_tile(shape, name):
                    t = psp.tile(shape, F32, tag=f"st{stc[0] % 3}", name=name)
                    stc[0] += 1
                    return t

                def proj_k(ch):
                    sl = slice(ch * 512, (ch + 1) * 512)
                    xtk = xs.tile([P, 4, 512], BF16, tag="xtk")
                    nc.sync.dma_start(xtk, xkv[:, :, sl])
                    kps = st_tile([P, 512], "kps")
                    for dk in range(4):
                        nc.tensor.matmul(kps, lhsT=wk_sb[:, dk, :],
                                         rhs=xtk[:, dk, :],
                                         start=(dk == 0), stop=(dk == 3))
                    nc.vector.tensor_scalar(kpt_t[ch][:], kps, 1.0, bk_sb,
                                            ALU.mult, ALU.add)

                def proj_v(ch):
                    sl = slice(ch * 512, (ch + 1) * 512)
                    xtv = xs.tile([P, 4, 512], BF16, tag="xtv")
                    nc.gpsimd.dma_start(xtv, xvv[:, :, sl])
                    for js in range(4):
                        psv = st_tile([P, P], "psv")
                        for dk in range(4):
                            nc.tensor.matmul(psv,
                                             lhsT=xtv[:, dk, js * P:(js + 1) * P],
                                             rhs=wv_sb[:, dk, :],
                                             start=(dk == 0), stop=False)
                        nc.tensor.matmul(psv, lhsT=onesr, rhs=bvr_sb,
                                         start=False, stop=True)
                        nc.vector.tensor_copy(vp_t[ch][:, js, 0:64],
                                              psv[:, 0:64])
                        nc.vector.tensor_copy(vp_t[ch][:, js, 65:129],
                                              psv[:, 64:128])

                def proj_q(ch):
                    sl = slice(ch * 512, (ch + 1) * 512)
                    xtq = xs.tile([P, 4, 512], BF16, tag="xtq")
                    nc.gpsimd.dma_start(xtq, xqv[:, :, sl])
                    qps = st_tile([P, 512], "qps")
                    for dk in range(4):
                        nc.tensor.matmul(qps, lhsT=wq_sb[:, dk, :],
                                         rhs=xtq[:, dk, :],
                                         start=(dk == 0), stop=(dk == 3))
                    nc.vector.tensor_scalar(qpt_t[ch][:], qps, 0.125, bq_sb,
                                            ALU.mult, ALU.add)

                # prologue: just chunk 0; the rest interleaves with
                # attention (k/v chunk c lands 2 pairs before first use)
                proj_k(0)
                proj_q(0)
                proj_v(0)
                wo_sb = const.tile([P, D], F32R, tag="wo")
                nc.sync.dma_start(wo_sb, wo)

                # Schraudolph fast-exp in bf16: exp(x) ~=
                # bitcast_bf16(int16(x * 2^7/ln2 + (127*2^7 - 7.42)))
                SC1, SC2 = 184.6650292, 16249.0

                def emit_pv(pair, ests, pv_ps):
                    for h in range(2):
                        for r in range(2):
                            j = 2 * pair + r
                            nc.tensor.matmul(
                                pv_ps[h][0:65, :],
                                lhsT=vp_t[j // 4][:, j % 4,
                                                  h * 65:(h + 1) * 65],
                                rhs=ests[h][:, r, :],
                                start=(j == 0), stop=(j == 31))

                for lqc in range(8):
                    pv_ps = [psp.tile([P, 512], F32, tag=f"pv{h}",
                                      name=f"pv{h}") for h in range(2)]
                    prev = None
                    for b in range(8):
                        for pair in range(2 * b, 2 * b + 2):
                            ests = []
                            for h in range(2):
                                hb = h * 64
                                st = st_tile([P, 2, 512], "st")
                                for r in range(2):
                                    j = 2 * pair + r
                                    nc.tensor.matmul(
                                        st[:, r, :],
                                        lhsT=kpt_t[j // 4][hb:hb + 64,
                                                           (j % 4) * P:(j % 4 + 1) * P],
                                        rhs=qpt_t[lqc][hb:hb + 64, :],
                                        start=True, stop=True)
                                est = esp.tile([P, 2, 512], BF16,
                                               tag=f"est{h}")
                                if (lqc >= V_OFFL_LQC and h == 1
                                        and pair % 2 == 1):
                                    nc.vector.tensor_scalar(
                                        est[:, :, :].bitcast(I16),
                                        st, SC1, SC2, ALU.mult, ALU.add)
                                else:
                                    nc.scalar.activation(est, st, AF.Exp)
                                ests.append(est)
                            if prev is not None:
                                emit_pv(prev[0], prev[1], pv_ps)
                            prev = (pair, ests)
                            # interleaved projections, half-chunk granular:
                            # k(c)/v(c) emitted at pairs 2(c-1), 2(c-1)+1
                            if lqc == 0 and pair < 14:
                                if pair % 2 == 0:
                                    proj_k(pair // 2 + 1)
                                else:
                                    proj_v(pair // 2 + 1)
                            elif lqc == 0 and pair == 14:
                                proj_q(1)
                            elif lqc == 0 and pair == 15:
                                proj_q(2)
                        if lqc in (1, 2) and b in (0, 4):
                            qch = 2 * lqc + 1 + (b // 4)
                            if 2 < qch < 8:
                                proj_q(qch)
                        if lqc == 3 and b == 0:
                            proj_q(7)
                    emit_pv(prev[0], prev[1], pv_ps)
                    # free pv banks fast: raw copy to SBUF; normalization
                    # (recip/broadcast/mult into mix) happens lazily and
                    # overlaps the next window's attention.
                    for h in range(2):
                        raw = bcp.tile([P, 512], F32, tag=f"raw{h}", bufs=2)
                        nc.vector.tensor_copy(raw[0:65, :], pv_ps[h][0:65, :])
                        rr = bcp.tile([1, 512], F32, tag=f"rr{h}", bufs=2)
                        nc.vector.reciprocal(rr, raw[64:65, :])
                        bc = bcp.tile([P, 512], F32, tag=f"bc{h}", bufs=2)
                        nc.gpsimd.partition_broadcast(bc, rr)
                        nc.vector.tensor_tensor(
                            mix_t[lqc][h * 64:(h + 1) * 64, :],
                            raw[0:64, :], bc[0:64, :], ALU.mult)

                # tail: out-projection. mix rows 0:64 = h0 c-dims,
                # 64:128 = h1 c-dims, so a single K=128 matmul against
                # wo_sb sums both heads' contributions.
                tags = ["st0", "st1", "st2", "pv0", "pv1"]
                for lqc in range(8):
                    for s in range(4):
                        i = lqc * 4 + s
                        ops = psp.tile([P, D], F32, tag=tags[i % 5],
                                       name="ops")
                        nc.tensor.matmul(
                            ops, lhsT=mix_t[lqc][:, s * P:(s + 1) * P],
                            rhs=wo_sb, start=True, stop=True)
                        ob = obp.tile([P, D], BF16, tag=f"ob{i % 3}")
                        if i % 2 == 0:
                            nc.scalar.copy(ob, ops)
                        else:
                            nc.vector.tensor_copy(ob, ops)
                        nc.sync.dma_start(
                            out[lqc * 512 + s * P:lqc * 512 + (s + 1) * P, :],
                            ob)

    nc.compile()
    return nc


def get_nc():
    global _NC
    if _NC is None:
        _NC = build()
    return _NC


def make_in_maps(q, k, v, Wq, bq, Wk, bk, Wv, bv, Wo, bo):
    bf = ml_dtypes.bfloat16
    q = np.asarray(q, np.float32)
    k = np.asarray(k, np.float32)
    v = np.asarray(v, np.float32)
    Wq = np.asarray(Wq, np.float32)
    Wk = np.asarray(Wk, np.float32)
    Wv = np.asarray(Wv, np.float32)
    Wo = np.asarray(Wo, np.float32)
    bq = np.asarray(bq, np.float32)
    bk = np.asarray(bk, np.float32)
    bv = np.asarray(bv, np.float32)
    xts = {}
    for n in range(2):
        xts[n] = (np.ascontiguousarray(q[n].T).astype(bf),
                  np.ascontiguousarray(k[n].T).astype(bf),
                  np.ascontiguousarray(v[n].T).astype(bf))
    in_maps = []
    for c in range(8):
        n, hp = c // 4, c % 4
        sl = slice(P * hp, P * (hp + 1))
        xq, xk, xv = xts[n]
        in_maps.append({
            "xqt": xq, "xkt": xk, "xvt": xv,
            "wq": np.ascontiguousarray(Wq[:, sl]).astype(bf),
            "wk": np.ascontiguousarray(Wk[:, sl]).astype(bf),
            "wv": np.ascontiguousarray(Wv[:, sl]).astype(bf),
            "wo": np.ascontiguousarray(Wo[sl, :]),
            "bqs": (bq[sl] * 0.125).reshape(P, 1).astype(np.float32),
            "bkc": bk[sl].reshape(P, 1).astype(np.float32),
            "bvr": bv[sl].reshape(1, P).astype(bf),
        })
    return in_maps


def assemble(results, bo):
    bo = np.asarray(bo, np.float32)
    out = np.zeros((2, L, D), np.float32)
    for c in range(8):
        out[c // 4] += np.asarray(results[c]["out"], dtype=np.float32)
    out += bo[None, None, :]
    return out


def kernel(q, k, v, Wq, bq, Wk, bk, Wv, bv, Wo, bo):
    nc = get_nc()
    in_maps = make_in_maps(q, k, v, Wq, bq, Wk, bk, Wv, bv, Wo, bo)
    res = bass_utils.run_bass_kernel_spmd(nc, in_maps, core_ids=list(range(8)))
    return assemble(res.results, bo)


if __name__ == "__main__":
    build()
    print("build ok")
